# revision 13
# baseline (speedup 1.0000x reference)
"""Trainium2 Bass kernel for nn_BPNN (moe_routing).

Strategy (sharding_hint: data-parallel over atoms):
  - Host routes atoms by element (MoE routing) and deals them round-robin
    to 8 cores so each core holds ~2500 atoms per element, padded to a
    fixed 3072-slot block per element (18 tiles of 512 atoms per core).
  - Each NeuronCore runs the full per-expert MLP forward + backward in
    fp32r (full-rate TF32-class matmuls) on its atom shard, producing the
    per-atom energies o and the per-atom fingerprint gradient dE (stored
    transposed [D=128, slots]).
  - Host unshards: per-image energy segment-sum, and the sparse COO
    fprime^T @ dE contraction (bincount over the 2M static nonzeros)
    against the device-computed dE.
"""

import sys

for _p in ("/opt/trn_rl_repo", "/root/.axon_site/_ro/trn_rl_repo"):
    if _p not in sys.path:
        sys.path.insert(0, _p)

import numpy as np

N_ATOMS = 60000
D = 128
H = 512
E = 3
N_IMAGES = 600
N_CORES = 8
PER_E = 2560          # per-core capacity per element (5 tiles of 512)
A_CAP = PER_E * E     # 9216 slots per core
TILE = 512
TILES_PER_E = PER_E // TILE
N_TILES = A_CAP // TILE
KT = H // 128         # 4 k-tiles over the hidden dim

_compiled = None
TRACE = False
LAST_RES = None


def _build_bass():
    import concourse.bacc as bacc
    import concourse.mybir as mybir
    from concourse.tile import TileContext

    F32 = mybir.dt.float32
    F32R = mybir.dt.float32r
    Act = mybir.ActivationFunctionType
    Alu = mybir.AluOpType

    nc = bacc.Bacc("TRN2", target_bir_lowering=False, debug=False,
                   num_devices=N_CORES)

    fpT = nc.dram_tensor('fpT', [D, A_CAP], F32R, kind="ExternalInput").ap()
    W1all = nc.dram_tensor('W1all', [128, E * H], F32R, kind="ExternalInput").ap()
    W2all = nc.dram_tensor('W2all', [128, E * KT * H], F32R, kind="ExternalInput").ap()
    W2Tall = nc.dram_tensor('W2Tall', [128, E * KT * H], F32R, kind="ExternalInput").ap()
    W3all = nc.dram_tensor('W3all', [128, E * KT], F32R, kind="ExternalInput").ap()
    W1Tall = nc.dram_tensor('W1Tall', [128, E * KT * D], F32R, kind="ExternalInput").ap()
    b1all = nc.dram_tensor('b1all', [128, E * KT], F32, kind="ExternalInput").ap()
    b2all = nc.dram_tensor('b2all', [128, E * KT], F32, kind="ExternalInput").ap()
    b3all = nc.dram_tensor('b3all', [1, E], F32, kind="ExternalInput").ap()
    w3col = nc.dram_tensor('w3col', [128, E * KT], F32, kind="ExternalInput").ap()

    dET = nc.dram_tensor('dET', [D, A_CAP], F32, kind="ExternalOutput").ap()
    o_out = nc.dram_tensor('o_out', [1, A_CAP], F32, kind="ExternalOutput").ap()

    with TileContext(nc) as tc:
        with tc.tile_pool(name="const", bufs=1) as cp, \
             tc.tile_pool(name="acts", bufs=2) as ap2, \
             tc.tile_pool(name="acts1", bufs=2) as ap1, \
             tc.tile_pool(name="psum", bufs=2, space="PSUM") as pp, \
             tc.tile_pool(name="psum1", bufs=1, space="PSUM") as pq:

            fpT_s = cp.tile([D, A_CAP], F32R)
            nc.sync.dma_start(out=fpT_s[:], in_=fpT[:])
            W1_s = cp.tile([128, E * H], F32R)
            nc.sync.dma_start(out=W1_s[:], in_=W1all[:])
            W2_s = cp.tile([128, E * KT * H], F32R)
            nc.sync.dma_start(out=W2_s[:], in_=W2all[:])
            W2T_s = cp.tile([128, E * KT * H], F32R)
            nc.sync.dma_start(out=W2T_s[:], in_=W2Tall[:])
            W3_s = cp.tile([128, E * KT], F32R)
            nc.sync.dma_start(out=W3_s[:], in_=W3all[:])
            W1T_s = cp.tile([128, E * KT * D], F32R)
            nc.sync.dma_start(out=W1T_s[:], in_=W1Tall[:])
            b1_s = cp.tile([128, E * KT], F32)
            nc.sync.dma_start(out=b1_s[:], in_=b1all[:])
            b2_s = cp.tile([128, E * KT], F32)
            nc.sync.dma_start(out=b2_s[:], in_=b2all[:])
            b3_s = cp.tile([1, E], F32)
            nc.sync.dma_start(out=b3_s[:], in_=b3all[:])
            w3c_s = cp.tile([128, E * KT], F32)
            nc.sync.dma_start(out=w3c_s[:], in_=w3col[:])



            for t in range(N_TILES):
                e = t // TILES_PER_E
                a0 = t * TILE
                asl = slice(a0, a0 + TILE)

                # ---- layer 1 forward: h1 = tanh(fp @ W1 + b1), transposed
                h1T = ap2.tile([128, KT * TILE], F32R, tag="h1T")
                for ch in range(KT):
                    ps = pp.tile([128, TILE], F32, tag="ps_a")
                    nc.tensor.matmul(
                        out=ps[:],
                        lhsT=W1_s[:, e * H + ch * 128: e * H + (ch + 1) * 128],
                        rhs=fpT_s[:, asl],
                        start=True, stop=True)
                    nc.scalar.activation(
                        out=h1T[:, ch * TILE:(ch + 1) * TILE], in_=ps[:],
                        func=Act.Tanh,
                        bias=b1_s[:, e * KT + ch: e * KT + ch + 1], scale=1.0)

                # ---- layer 2 forward: h2 = tanh(h1 @ W2 + b2), transposed
                h2T = ap2.tile([128, KT * TILE], F32R, tag="h2T")
                for ch in range(KT):
                    ps = pp.tile([128, TILE], F32, tag="ps_b")
                    for kt in range(KT):
                        off = (e * KT + kt) * H + ch * 128
                        nc.tensor.matmul(
                            out=ps[:],
                            lhsT=W2_s[:, off: off + 128],
                            rhs=h1T[:, kt * TILE:(kt + 1) * TILE],
                            start=(kt == 0), stop=(kt == KT - 1))
                    nc.scalar.activation(
                        out=h2T[:, ch * TILE:(ch + 1) * TILE], in_=ps[:],
                        func=Act.Tanh,
                        bias=b2_s[:, e * KT + ch: e * KT + ch + 1], scale=1.0)

                # ---- layer 3 forward: o = h2 @ W3 + b3  -> [1, TILE]
                ps_o = pq.tile([1, TILE], F32, tag="ps_o")
                for kt in range(KT):
                    nc.tensor.matmul(
                        out=ps_o[:],
                        lhsT=W3_s[:, e * KT + kt: e * KT + kt + 1],
                        rhs=h2T[:, kt * TILE:(kt + 1) * TILE],
                        start=(kt == 0), stop=(kt == KT - 1))
                o_t = ap2.tile([1, TILE], F32, tag="o_t")
                nc.vector.tensor_copy(out=o_t[:], in_=ps_o[:])
                nc.sync.dma_start(out=o_out[0:1, asl], in_=o_t[:])

                # ---- backward: g2' = w3*(h2^2-1) = w3*(h2+1)*(h2-1) = -dE/da2
                tA = ap1.tile([128, KT * TILE], F32, tag="tA")
                nc.vector.scalar_tensor_tensor(
                    out=tA[:].rearrange("p (k t) -> p k t", k=KT),
                    in0=h2T[:].bitcast(F32).rearrange("p (k t) -> p k t", k=KT),
                    scalar=1.0,
                    in1=w3c_s[:, e * KT:(e + 1) * KT][:, :, None]
                        .to_broadcast([128, KT, TILE]),
                    op0=Alu.add, op1=Alu.mult)
                g2p = ap1.tile([128, KT * TILE], F32R, tag="g2p")
                nc.vector.scalar_tensor_tensor(
                    out=g2p[:], in0=h2T[:].bitcast(F32), scalar=1.0,
                    in1=tA[:], op0=Alu.subtract, op1=Alu.mult)

                # dh1' = W2 @ g2' (= -dh1), then g1'' = (h1^2 - 1) * dh1' = +dE/da1
                g1 = ap1.tile([128, KT * TILE], F32R, tag="g1")
                for ch in range(KT):
                    ps = pp.tile([128, TILE], F32, tag="ps_c")
                    for kt in range(KT):
                        off = (e * KT + kt) * H + ch * 128
                        nc.tensor.matmul(
                            out=ps[:],
                            lhsT=W2T_s[:, off: off + 128],
                            rhs=g2p[:, kt * TILE:(kt + 1) * TILE],
                            start=(kt == 0), stop=(kt == KT - 1))
                    tB = ap2.tile([128, TILE], F32, tag="tB")
                    nc.vector.scalar_tensor_tensor(
                        out=tB[:],
                        in0=h1T[:, ch * TILE:(ch + 1) * TILE].bitcast(F32),
                        scalar=1.0, in1=ps[:],
                        op0=Alu.add, op1=Alu.mult)
                    nc.vector.scalar_tensor_tensor(
                        out=g1[:, ch * TILE:(ch + 1) * TILE],
                        in0=h1T[:, ch * TILE:(ch + 1) * TILE].bitcast(F32),
                        scalar=1.0, in1=tB[:],
                        op0=Alu.subtract, op1=Alu.mult)

                # dfpT = W1 @ g1  -> dE tile [D, TILE]
                ps_d = pq.tile([128, TILE], F32, tag="ps_d")
                for kt in range(KT):
                    nc.tensor.matmul(
                        out=ps_d[:],
                        lhsT=W1T_s[:, (e * KT + kt) * D: (e * KT + kt + 1) * D],
                        rhs=g1[:, kt * TILE:(kt + 1) * TILE],
                        start=(kt == 0), stop=(kt == KT - 1))
                dE_t = ap2.tile([128, TILE], F32, tag="dE_t")
                nc.vector.tensor_copy(out=dE_t[:], in_=ps_d[:])
                nc.sync.dma_start(out=dET[:, asl], in_=dE_t[:])

    nc.compile()
    return nc


def kernel(fingerprints, atomic_numbers, image_idx, fprime_rows, fprime_cols,
           fprime_vals, W1, b1, W2, b2, W3, b3):
    global _compiled
    from concourse.bass_utils import run_bass_kernel_spmd

    fingerprints = np.asarray(fingerprints, dtype=np.float32)
    atomic_numbers = np.asarray(atomic_numbers, dtype=np.int32)
    image_idx = np.asarray(image_idx, dtype=np.int32)
    fprime_rows = np.asarray(fprime_rows, dtype=np.int64)
    fprime_cols = np.asarray(fprime_cols, dtype=np.int64)
    fprime_vals = np.asarray(fprime_vals, dtype=np.float32)
    W1 = np.asarray(W1, dtype=np.float32)
    b1 = np.asarray(b1, dtype=np.float32)
    W2 = np.asarray(W2, dtype=np.float32)
    b2 = np.asarray(b2, dtype=np.float32)
    W3 = np.asarray(W3, dtype=np.float32)
    b3 = np.asarray(b3, dtype=np.float32)

    # ---- route atoms: sort by element, deal round-robin to cores
    core_of = np.empty(N_ATOMS, np.int32)
    slot_of = np.empty(N_ATOMS, np.int32)
    per_core_atoms = [[] for _ in range(N_CORES)]
    for e in range(E):
        atoms_e = np.nonzero(atomic_numbers == e)[0]
        for m in range(N_CORES):
            chunk = atoms_e[m::N_CORES]
            assert len(chunk) <= PER_E, f"element {e} core {m}: {len(chunk)}"
            core_of[chunk] = m
            slot_of[chunk] = e * PER_E + np.arange(len(chunk), dtype=np.int32)
            per_core_atoms[m].append(chunk)

    # ---- per-core fpT
    in_maps = []
    shared = {}
    # weight packing (shared by all cores)
    W1all = np.zeros((128, E * H), np.float32)
    W2all = np.zeros((128, E * KT * H), np.float32)
    W2Tall = np.zeros((128, E * KT * H), np.float32)
    W3all = np.zeros((128, E * KT), np.float32)
    W1Tall = np.zeros((128, E * KT * D), np.float32)
    b1all = np.zeros((128, E * KT), np.float32)
    b2all = np.zeros((128, E * KT), np.float32)
    b3all = np.zeros((1, E), np.float32)
    w3colp = np.zeros((128, E * KT), np.float32)
    for e in range(E):
        W1all[:, e * H:(e + 1) * H] = W1[e]                      # [D, H]
        for kt in range(KT):
            W2all[:, (e * KT + kt) * H:(e * KT + kt + 1) * H] = \
                W2[e][kt * 128:(kt + 1) * 128, :]
            W2Tall[:, (e * KT + kt) * H:(e * KT + kt + 1) * H] = \
                W2[e][:, kt * 128:(kt + 1) * 128].T
            W3all[:, e * KT + kt] = W3[e][kt * 128:(kt + 1) * 128, 0]
            w3colp[:, e * KT + kt] = W3[e][kt * 128:(kt + 1) * 128, 0]
            W1Tall[:, (e * KT + kt) * D:(e * KT + kt + 1) * D] = \
                W1[e][:, kt * 128:(kt + 1) * 128].T
            b1all[:, e * KT + kt] = b1[e][kt * 128:(kt + 1) * 128]
            b2all[:, e * KT + kt] = b2[e][kt * 128:(kt + 1) * 128]
        b3all[0, e] = b3[e, 0]
    shared = dict(W1all=W1all, W2all=W2all, W2Tall=W2Tall, W3all=W3all,
                  W1Tall=W1Tall, b1all=b1all, b2all=b2all, b3all=b3all,
                  w3col=w3colp)

    for m in range(N_CORES):
        fpT_m = np.zeros((D, A_CAP), np.float32)
        for e in range(E):
            chunk = per_core_atoms[m][e]
            fpT_m[:, e * PER_E: e * PER_E + len(chunk)] = fingerprints[chunk].T
        in_maps.append(dict(fpT=fpT_m, **shared))

    if _compiled is None:
        _compiled = _build_bass()
    nc = _compiled

    res = run_bass_kernel_spmd(nc, in_maps, list(range(N_CORES)), trace=TRACE)
    globals()['LAST_RES'] = res

    dET_all = np.stack([res.results[m]['dET'] for m in range(N_CORES)])  # [8,128,A_CAP]
    o_all = np.stack([res.results[m]['o_out'][0] for m in range(N_CORES)])  # [8,A_CAP]

    # ---- unshard energies
    o_atom = o_all[core_of, slot_of[np.arange(N_ATOMS)]]
    o_atom = o_atom + b3[atomic_numbers, 0]
    energy = np.bincount(image_idx, weights=o_atom.astype(np.float64),
                         minlength=N_IMAGES).astype(np.float32)

    # ---- unshard forces: forces = -fprime^T @ dE
    r_atom = fprime_rows // D
    r_comp = fprime_rows % D
    dE_at_rows = dET_all[core_of[r_atom], r_comp, slot_of[r_atom]]
    w = fprime_vals.astype(np.float64) * dE_at_rows
    forces_flat = -np.bincount(fprime_cols, weights=w, minlength=3 * N_ATOMS)
    forces = forces_flat.reshape(-1, 3).astype(np.float32)

    latent = np.zeros((0,), np.float32)
    return energy, forces, latent


# revision 14
# speedup vs baseline: 1.1197x; 1.1197x over previous
"""Trainium2 Bass kernel for nn_BPNN (moe_routing).

Strategy (sharding_hint: data-parallel over atoms):
  - Host routes atoms by element (MoE routing) and deals them round-robin
    to 8 cores so each core holds ~2500 atoms per element, padded to a
    fixed 3072-slot block per element (18 tiles of 512 atoms per core).
  - Each NeuronCore runs the full per-expert MLP forward + backward in
    fp32r (full-rate TF32-class matmuls) on its atom shard, producing the
    per-atom energies o and the per-atom fingerprint gradient dE (stored
    transposed [D=128, slots]).
  - Host unshards: per-image energy segment-sum, and the sparse COO
    fprime^T @ dE contraction (bincount over the 2M static nonzeros)
    against the device-computed dE.
"""

import sys

for _p in ("/opt/trn_rl_repo", "/root/.axon_site/_ro/trn_rl_repo"):
    if _p not in sys.path:
        sys.path.insert(0, _p)

import numpy as np

N_ATOMS = 60000
D = 128
H = 512
E = 3
N_IMAGES = 600
N_CORES = 8
PER_E = 2560          # per-core capacity per element (5 tiles of 512)
A_CAP = PER_E * E     # 9216 slots per core
TILE = 512
TILES_PER_E = PER_E // TILE
N_TILES = A_CAP // TILE
KT = H // 128         # 4 k-tiles over the hidden dim

_compiled = None
TRACE = False
LAST_RES = None


def _build_bass():
    import concourse.bacc as bacc
    import concourse.mybir as mybir
    from concourse.tile import TileContext

    F32 = mybir.dt.float32
    F32R = mybir.dt.float32r
    Act = mybir.ActivationFunctionType
    Alu = mybir.AluOpType

    nc = bacc.Bacc("TRN2", target_bir_lowering=False, debug=False,
                   num_devices=N_CORES)

    fpT = nc.dram_tensor('fpT', [D, A_CAP], F32R, kind="ExternalInput").ap()
    W1all = nc.dram_tensor('W1all', [128, E * H], F32R, kind="ExternalInput").ap()
    W2all = nc.dram_tensor('W2all', [128, E * KT * H], F32R, kind="ExternalInput").ap()
    W2Tall = nc.dram_tensor('W2Tall', [128, E * KT * H], F32R, kind="ExternalInput").ap()
    W3all = nc.dram_tensor('W3all', [128, E * KT], F32R, kind="ExternalInput").ap()
    W1Tall = nc.dram_tensor('W1Tall', [128, E * KT * D], F32R, kind="ExternalInput").ap()
    b1all = nc.dram_tensor('b1all', [128, E * KT], F32, kind="ExternalInput").ap()
    b2all = nc.dram_tensor('b2all', [128, E * KT], F32, kind="ExternalInput").ap()
    b3all = nc.dram_tensor('b3all', [1, E], F32, kind="ExternalInput").ap()
    w3col = nc.dram_tensor('w3col', [128, E * KT], F32, kind="ExternalInput").ap()

    dET = nc.dram_tensor('dET', [D, A_CAP], F32, kind="ExternalOutput").ap()
    o_out = nc.dram_tensor('o_out', [1, A_CAP], F32, kind="ExternalOutput").ap()

    with TileContext(nc) as tc:
        with tc.tile_pool(name="const", bufs=1) as cp, \
             tc.tile_pool(name="acts", bufs=2) as ap2, \
             tc.tile_pool(name="acts1", bufs=2) as ap1, \
             tc.tile_pool(name="psum", bufs=2, space="PSUM") as pp, \
             tc.tile_pool(name="psum1", bufs=1, space="PSUM") as pq:

            fpT_s = cp.tile([D, A_CAP], F32R)
            nc.sync.dma_start(out=fpT_s[:], in_=fpT[:])
            W1_s = cp.tile([128, E * H], F32R)
            nc.sync.dma_start(out=W1_s[:], in_=W1all[:])
            W2_s = cp.tile([128, E * KT * H], F32R)
            nc.sync.dma_start(out=W2_s[:], in_=W2all[:])
            W2T_s = cp.tile([128, E * KT * H], F32R)
            nc.sync.dma_start(out=W2T_s[:], in_=W2Tall[:])
            W3_s = cp.tile([128, E * KT], F32R)
            nc.sync.dma_start(out=W3_s[:], in_=W3all[:])
            W1T_s = cp.tile([128, E * KT * D], F32R)
            nc.sync.dma_start(out=W1T_s[:], in_=W1Tall[:])
            b1_s = cp.tile([128, E * KT], F32)
            nc.sync.dma_start(out=b1_s[:], in_=b1all[:])
            b2_s = cp.tile([128, E * KT], F32)
            nc.sync.dma_start(out=b2_s[:], in_=b2all[:])
            b3_s = cp.tile([1, E], F32)
            nc.sync.dma_start(out=b3_s[:], in_=b3all[:])
            w3c_s = cp.tile([128, E * KT], F32)
            nc.sync.dma_start(out=w3c_s[:], in_=w3col[:])



            for t in range(N_TILES):
                e = t // TILES_PER_E
                a0 = t * TILE
                asl = slice(a0, a0 + TILE)

                # ---- layer 1 forward: h1 = tanh(fp @ W1 + b1), transposed
                h1T = ap2.tile([128, KT * TILE], F32R, tag="h1T")
                for ch in range(KT):
                    ps = pp.tile([128, TILE], F32, tag="ps_a")
                    nc.tensor.matmul(
                        out=ps[:],
                        lhsT=W1_s[:, e * H + ch * 128: e * H + (ch + 1) * 128],
                        rhs=fpT_s[:, asl],
                        start=True, stop=True)
                    nc.scalar.activation(
                        out=h1T[:, ch * TILE:(ch + 1) * TILE], in_=ps[:],
                        func=Act.Tanh,
                        bias=b1_s[:, e * KT + ch: e * KT + ch + 1], scale=1.0)

                # ---- layer 2 forward: h2 = tanh(h1 @ W2 + b2), transposed
                h2T = ap2.tile([128, KT * TILE], F32R, tag="h2T")
                for ch in range(KT):
                    ps = pp.tile([128, TILE], F32, tag="ps_b")
                    for kt in range(KT):
                        off = (e * KT + kt) * H + ch * 128
                        nc.tensor.matmul(
                            out=ps[:],
                            lhsT=W2_s[:, off: off + 128],
                            rhs=h1T[:, kt * TILE:(kt + 1) * TILE],
                            start=(kt == 0), stop=(kt == KT - 1))
                    nc.scalar.activation(
                        out=h2T[:, ch * TILE:(ch + 1) * TILE], in_=ps[:],
                        func=Act.Tanh,
                        bias=b2_s[:, e * KT + ch: e * KT + ch + 1], scale=1.0)

                # ---- layer 3 forward: o = h2 @ W3 + b3  -> [1, TILE]
                ps_o = pq.tile([1, TILE], F32, tag="ps_o")
                for kt in range(KT):
                    nc.tensor.matmul(
                        out=ps_o[:],
                        lhsT=W3_s[:, e * KT + kt: e * KT + kt + 1],
                        rhs=h2T[:, kt * TILE:(kt + 1) * TILE],
                        start=(kt == 0), stop=(kt == KT - 1))
                o_t = ap2.tile([1, TILE], F32, tag="o_t")
                nc.vector.tensor_copy(out=o_t[:], in_=ps_o[:])
                nc.sync.dma_start(out=o_out[0:1, asl], in_=o_t[:])

                # ---- backward.  s2 = h2^2 - 1 = -tanh'(a2)
                # g2' = w3*(h2^2 - 1) = -dE/da2
                h2sq = ap1.tile([128, KT * TILE], F32, tag="h2sq")
                nc.scalar.activation(out=h2sq[:], in_=h2T[:].bitcast(F32),
                                     func=Act.Square)
                h1sq = ap1.tile([128, KT * TILE], F32, tag="h1sq")
                nc.scalar.activation(out=h1sq[:], in_=h1T[:].bitcast(F32),
                                     func=Act.Square)
                g2p = ap1.tile([128, KT * TILE], F32R, tag="g2p")
                nc.vector.scalar_tensor_tensor(
                    out=g2p[:].rearrange("p (k t) -> p k t", k=KT),
                    in0=h2sq[:].rearrange("p (k t) -> p k t", k=KT),
                    scalar=1.0,
                    in1=w3c_s[:, e * KT:(e + 1) * KT][:, :, None]
                        .to_broadcast([128, KT, TILE]),
                    op0=Alu.subtract, op1=Alu.mult)

                # dh1' = W2 @ g2' (= -dh1), then g1'' = (h1^2 - 1) * dh1' = +dE/da1
                g1 = ap1.tile([128, KT * TILE], F32R, tag="g1")
                for ch in range(KT):
                    ps = pp.tile([128, TILE], F32, tag="ps_c")
                    for kt in range(KT):
                        off = (e * KT + kt) * H + ch * 128
                        nc.tensor.matmul(
                            out=ps[:],
                            lhsT=W2T_s[:, off: off + 128],
                            rhs=g2p[:, kt * TILE:(kt + 1) * TILE],
                            start=(kt == 0), stop=(kt == KT - 1))
                    nc.vector.scalar_tensor_tensor(
                        out=g1[:, ch * TILE:(ch + 1) * TILE],
                        in0=h1sq[:, ch * TILE:(ch + 1) * TILE],
                        scalar=1.0,
                        in1=ps[:],
                        op0=Alu.subtract, op1=Alu.mult)

                # dfpT = W1 @ g1  -> dE tile [D, TILE]
                ps_d = pq.tile([128, TILE], F32, tag="ps_d")
                for kt in range(KT):
                    nc.tensor.matmul(
                        out=ps_d[:],
                        lhsT=W1T_s[:, (e * KT + kt) * D: (e * KT + kt + 1) * D],
                        rhs=g1[:, kt * TILE:(kt + 1) * TILE],
                        start=(kt == 0), stop=(kt == KT - 1))
                dE_t = ap2.tile([128, TILE], F32, tag="dE_t")
                nc.vector.tensor_copy(out=dE_t[:], in_=ps_d[:])
                nc.sync.dma_start(out=dET[:, asl], in_=dE_t[:])

    nc.compile()
    return nc


def kernel(fingerprints, atomic_numbers, image_idx, fprime_rows, fprime_cols,
           fprime_vals, W1, b1, W2, b2, W3, b3):
    global _compiled
    from concourse.bass_utils import run_bass_kernel_spmd

    fingerprints = np.asarray(fingerprints, dtype=np.float32)
    atomic_numbers = np.asarray(atomic_numbers, dtype=np.int32)
    image_idx = np.asarray(image_idx, dtype=np.int32)
    fprime_rows = np.asarray(fprime_rows, dtype=np.int64)
    fprime_cols = np.asarray(fprime_cols, dtype=np.int64)
    fprime_vals = np.asarray(fprime_vals, dtype=np.float32)
    W1 = np.asarray(W1, dtype=np.float32)
    b1 = np.asarray(b1, dtype=np.float32)
    W2 = np.asarray(W2, dtype=np.float32)
    b2 = np.asarray(b2, dtype=np.float32)
    W3 = np.asarray(W3, dtype=np.float32)
    b3 = np.asarray(b3, dtype=np.float32)

    # ---- route atoms: sort by element, deal round-robin to cores
    core_of = np.empty(N_ATOMS, np.int32)
    slot_of = np.empty(N_ATOMS, np.int32)
    per_core_atoms = [[] for _ in range(N_CORES)]
    for e in range(E):
        atoms_e = np.nonzero(atomic_numbers == e)[0]
        for m in range(N_CORES):
            chunk = atoms_e[m::N_CORES]
            assert len(chunk) <= PER_E, f"element {e} core {m}: {len(chunk)}"
            core_of[chunk] = m
            slot_of[chunk] = e * PER_E + np.arange(len(chunk), dtype=np.int32)
            per_core_atoms[m].append(chunk)

    # ---- per-core fpT
    in_maps = []
    shared = {}
    # weight packing (shared by all cores)
    W1all = np.zeros((128, E * H), np.float32)
    W2all = np.zeros((128, E * KT * H), np.float32)
    W2Tall = np.zeros((128, E * KT * H), np.float32)
    W3all = np.zeros((128, E * KT), np.float32)
    W1Tall = np.zeros((128, E * KT * D), np.float32)
    b1all = np.zeros((128, E * KT), np.float32)
    b2all = np.zeros((128, E * KT), np.float32)
    b3all = np.zeros((1, E), np.float32)
    w3colp = np.zeros((128, E * KT), np.float32)
    for e in range(E):
        W1all[:, e * H:(e + 1) * H] = W1[e]                      # [D, H]
        for kt in range(KT):
            W2all[:, (e * KT + kt) * H:(e * KT + kt + 1) * H] = \
                W2[e][kt * 128:(kt + 1) * 128, :]
            W2Tall[:, (e * KT + kt) * H:(e * KT + kt + 1) * H] = \
                W2[e][:, kt * 128:(kt + 1) * 128].T
            W3all[:, e * KT + kt] = W3[e][kt * 128:(kt + 1) * 128, 0]
            w3colp[:, e * KT + kt] = W3[e][kt * 128:(kt + 1) * 128, 0]
            W1Tall[:, (e * KT + kt) * D:(e * KT + kt + 1) * D] = \
                W1[e][:, kt * 128:(kt + 1) * 128].T
            b1all[:, e * KT + kt] = b1[e][kt * 128:(kt + 1) * 128]
            b2all[:, e * KT + kt] = b2[e][kt * 128:(kt + 1) * 128]
        b3all[0, e] = b3[e, 0]
    shared = dict(W1all=W1all, W2all=W2all, W2Tall=W2Tall, W3all=W3all,
                  W1Tall=W1Tall, b1all=b1all, b2all=b2all, b3all=b3all,
                  w3col=w3colp)

    for m in range(N_CORES):
        fpT_m = np.zeros((D, A_CAP), np.float32)
        for e in range(E):
            chunk = per_core_atoms[m][e]
            fpT_m[:, e * PER_E: e * PER_E + len(chunk)] = fingerprints[chunk].T
        in_maps.append(dict(fpT=fpT_m, **shared))

    if _compiled is None:
        _compiled = _build_bass()
    nc = _compiled

    res = run_bass_kernel_spmd(nc, in_maps, list(range(N_CORES)), trace=TRACE)
    globals()['LAST_RES'] = res

    dET_all = np.stack([res.results[m]['dET'] for m in range(N_CORES)])  # [8,128,A_CAP]
    o_all = np.stack([res.results[m]['o_out'][0] for m in range(N_CORES)])  # [8,A_CAP]

    # ---- unshard energies
    o_atom = o_all[core_of, slot_of[np.arange(N_ATOMS)]]
    o_atom = o_atom + b3[atomic_numbers, 0]
    energy = np.bincount(image_idx, weights=o_atom.astype(np.float64),
                         minlength=N_IMAGES).astype(np.float32)

    # ---- unshard forces: forces = -fprime^T @ dE
    r_atom = fprime_rows // D
    r_comp = fprime_rows % D
    dE_at_rows = dET_all[core_of[r_atom], r_comp, slot_of[r_atom]]
    w = fprime_vals.astype(np.float64) * dE_at_rows
    forces_flat = -np.bincount(fprime_cols, weights=w, minlength=3 * N_ATOMS)
    forces = forces_flat.reshape(-1, 3).astype(np.float32)

    latent = np.zeros((0,), np.float32)
    return energy, forces, latent


# revision 15
# speedup vs baseline: 1.1368x; 1.0153x over previous
"""Trainium2 Bass kernel for nn_BPNN (moe_routing).

Strategy (sharding_hint: data-parallel over atoms):
  - Host routes atoms by element (MoE routing) and deals them round-robin
    to 8 cores so each core holds ~2500 atoms per element, padded to a
    fixed 3072-slot block per element (18 tiles of 512 atoms per core).
  - Each NeuronCore runs the full per-expert MLP forward + backward in
    fp32r (full-rate TF32-class matmuls) on its atom shard, producing the
    per-atom energies o and the per-atom fingerprint gradient dE (stored
    transposed [D=128, slots]).
  - Host unshards: per-image energy segment-sum, and the sparse COO
    fprime^T @ dE contraction (bincount over the 2M static nonzeros)
    against the device-computed dE.
"""

import sys

for _p in ("/opt/trn_rl_repo", "/root/.axon_site/_ro/trn_rl_repo"):
    if _p not in sys.path:
        sys.path.insert(0, _p)

import numpy as np

N_ATOMS = 60000
D = 128
H = 512
E = 3
N_IMAGES = 600
N_CORES = 8
PER_E = 2560          # per-core capacity per element (5 tiles of 512)
A_CAP = PER_E * E     # 9216 slots per core
TILE = 512
TILES_PER_E = PER_E // TILE
N_TILES = A_CAP // TILE
KT = H // 128         # 4 k-tiles over the hidden dim

_compiled = None
TRACE = False
LAST_RES = None


def _build_bass():
    import concourse.bacc as bacc
    import concourse.mybir as mybir
    from concourse.tile import TileContext

    F32 = mybir.dt.float32
    F32R = mybir.dt.float32r
    Act = mybir.ActivationFunctionType
    Alu = mybir.AluOpType

    nc = bacc.Bacc("TRN2", target_bir_lowering=False, debug=False,
                   num_devices=N_CORES)

    fpT = nc.dram_tensor('fpT', [D, A_CAP], F32R, kind="ExternalInput").ap()
    W1all = nc.dram_tensor('W1all', [128, E * H], F32R, kind="ExternalInput").ap()
    W2all = nc.dram_tensor('W2all', [128, E * KT * H], F32R, kind="ExternalInput").ap()
    W2Tall = nc.dram_tensor('W2Tall', [128, E * KT * H], F32R, kind="ExternalInput").ap()
    W3all = nc.dram_tensor('W3all', [128, E * KT], F32R, kind="ExternalInput").ap()
    W1Tall = nc.dram_tensor('W1Tall', [128, E * KT * D], F32R, kind="ExternalInput").ap()
    b1all = nc.dram_tensor('b1all', [128, E * KT], F32, kind="ExternalInput").ap()
    b2all = nc.dram_tensor('b2all', [128, E * KT], F32, kind="ExternalInput").ap()
    b3all = nc.dram_tensor('b3all', [1, E], F32, kind="ExternalInput").ap()
    w3col = nc.dram_tensor('w3col', [128, E * KT], F32, kind="ExternalInput").ap()

    dET = nc.dram_tensor('dET', [D, A_CAP], F32, kind="ExternalOutput").ap()
    o_out = nc.dram_tensor('o_out', [1, A_CAP], F32, kind="ExternalOutput").ap()

    with TileContext(nc) as tc:
        with tc.tile_pool(name="const", bufs=1) as cp, \
             tc.tile_pool(name="acts", bufs=2) as ap2, \
             tc.tile_pool(name="acts1", bufs=2) as ap1, \
             tc.tile_pool(name="acts3", bufs=3) as ap3, \
             tc.tile_pool(name="psum", bufs=2, space="PSUM") as pp, \
             tc.tile_pool(name="psum1", bufs=1, space="PSUM") as pq:

            fpT_s = cp.tile([D, A_CAP], F32R)
            nc.sync.dma_start(out=fpT_s[:], in_=fpT[:])
            W1_s = cp.tile([128, E * H], F32R)
            nc.sync.dma_start(out=W1_s[:], in_=W1all[:])
            W2_s = cp.tile([128, E * KT * H], F32R)
            nc.sync.dma_start(out=W2_s[:], in_=W2all[:])
            W2T_s = cp.tile([128, E * KT * H], F32R)
            nc.sync.dma_start(out=W2T_s[:], in_=W2Tall[:])
            W3_s = cp.tile([128, E * KT], F32R)
            nc.sync.dma_start(out=W3_s[:], in_=W3all[:])
            W1T_s = cp.tile([128, E * KT * D], F32R)
            nc.sync.dma_start(out=W1T_s[:], in_=W1Tall[:])
            b1_s = cp.tile([128, E * KT], F32)
            nc.sync.dma_start(out=b1_s[:], in_=b1all[:])
            b2_s = cp.tile([128, E * KT], F32)
            nc.sync.dma_start(out=b2_s[:], in_=b2all[:])
            b3_s = cp.tile([1, E], F32)
            nc.sync.dma_start(out=b3_s[:], in_=b3all[:])
            w3c_s = cp.tile([128, E * KT], F32)
            nc.sync.dma_start(out=w3c_s[:], in_=w3col[:])



            for t in range(N_TILES):
                e = t // TILES_PER_E
                a0 = t * TILE
                asl = slice(a0, a0 + TILE)

                # ---- layer 1 forward: h1 = tanh(fp @ W1 + b1), transposed
                h1T = ap3.tile([128, KT * TILE], F32R, tag="h1T")
                for ch in range(KT):
                    ps = pp.tile([128, TILE], F32, tag="ps_a")
                    nc.tensor.matmul(
                        out=ps[:],
                        lhsT=W1_s[:, e * H + ch * 128: e * H + (ch + 1) * 128],
                        rhs=fpT_s[:, asl],
                        start=True, stop=True)
                    nc.scalar.activation(
                        out=h1T[:, ch * TILE:(ch + 1) * TILE], in_=ps[:],
                        func=Act.Tanh,
                        bias=b1_s[:, e * KT + ch: e * KT + ch + 1], scale=1.0)

                # ---- layer 2 forward: h2 = tanh(h1 @ W2 + b2), transposed
                h2T = ap2.tile([128, KT * TILE], F32R, tag="h2T")
                for ch in range(KT):
                    ps = pp.tile([128, TILE], F32, tag="ps_b")
                    for kt in range(KT):
                        off = (e * KT + kt) * H + ch * 128
                        nc.tensor.matmul(
                            out=ps[:],
                            lhsT=W2_s[:, off: off + 128],
                            rhs=h1T[:, kt * TILE:(kt + 1) * TILE],
                            start=(kt == 0), stop=(kt == KT - 1))
                    nc.scalar.activation(
                        out=h2T[:, ch * TILE:(ch + 1) * TILE], in_=ps[:],
                        func=Act.Tanh,
                        bias=b2_s[:, e * KT + ch: e * KT + ch + 1], scale=1.0)

                # ---- layer 3 forward: o = h2 @ W3 + b3  -> [1, TILE]
                ps_o = pq.tile([1, TILE], F32, tag="ps_o")
                for kt in range(KT):
                    nc.tensor.matmul(
                        out=ps_o[:],
                        lhsT=W3_s[:, e * KT + kt: e * KT + kt + 1],
                        rhs=h2T[:, kt * TILE:(kt + 1) * TILE],
                        start=(kt == 0), stop=(kt == KT - 1))
                o_t = ap2.tile([1, TILE], F32, tag="o_t")
                nc.vector.tensor_copy(out=o_t[:], in_=ps_o[:])
                nc.sync.dma_start(out=o_out[0:1, asl], in_=o_t[:])

                # ---- backward.  s2 = h2^2 - 1 = -tanh'(a2)
                # g2' = w3*(h2^2 - 1) = -dE/da2
                h2sq = ap1.tile([128, KT * TILE], F32, tag="h2sq")
                nc.scalar.activation(out=h2sq[:], in_=h2T[:].bitcast(F32),
                                     func=Act.Square)
                h1sq = ap1.tile([128, KT * TILE], F32, tag="h1sq")
                nc.scalar.activation(out=h1sq[:], in_=h1T[:].bitcast(F32),
                                     func=Act.Square)
                g2p = ap1.tile([128, KT * TILE], F32R, tag="g2p")
                nc.vector.scalar_tensor_tensor(
                    out=g2p[:].rearrange("p (k t) -> p k t", k=KT),
                    in0=h2sq[:].rearrange("p (k t) -> p k t", k=KT),
                    scalar=1.0,
                    in1=w3c_s[:, e * KT:(e + 1) * KT][:, :, None]
                        .to_broadcast([128, KT, TILE]),
                    op0=Alu.subtract, op1=Alu.mult)

                # dh1' = W2 @ g2' (= -dh1), then g1'' = (h1^2 - 1) * dh1' = +dE/da1
                g1 = ap1.tile([128, KT * TILE], F32R, tag="g1")
                for ch in range(KT):
                    ps = pp.tile([128, TILE], F32, tag="ps_c")
                    for kt in range(KT):
                        off = (e * KT + kt) * H + ch * 128
                        nc.tensor.matmul(
                            out=ps[:],
                            lhsT=W2T_s[:, off: off + 128],
                            rhs=g2p[:, kt * TILE:(kt + 1) * TILE],
                            start=(kt == 0), stop=(kt == KT - 1))
                    nc.vector.scalar_tensor_tensor(
                        out=g1[:, ch * TILE:(ch + 1) * TILE],
                        in0=h1sq[:, ch * TILE:(ch + 1) * TILE],
                        scalar=1.0,
                        in1=ps[:],
                        op0=Alu.subtract, op1=Alu.mult)

                # dfpT = W1 @ g1  -> dE tile [D, TILE]
                ps_d = pq.tile([128, TILE], F32, tag="ps_d")
                for kt in range(KT):
                    nc.tensor.matmul(
                        out=ps_d[:],
                        lhsT=W1T_s[:, (e * KT + kt) * D: (e * KT + kt + 1) * D],
                        rhs=g1[:, kt * TILE:(kt + 1) * TILE],
                        start=(kt == 0), stop=(kt == KT - 1))
                dE_t = ap2.tile([128, TILE], F32, tag="dE_t")
                nc.vector.tensor_copy(out=dE_t[:], in_=ps_d[:])
                nc.sync.dma_start(out=dET[:, asl], in_=dE_t[:])

    nc.compile()
    return nc


def kernel(fingerprints, atomic_numbers, image_idx, fprime_rows, fprime_cols,
           fprime_vals, W1, b1, W2, b2, W3, b3):
    global _compiled
    from concourse.bass_utils import run_bass_kernel_spmd

    fingerprints = np.asarray(fingerprints, dtype=np.float32)
    atomic_numbers = np.asarray(atomic_numbers, dtype=np.int32)
    image_idx = np.asarray(image_idx, dtype=np.int32)
    fprime_rows = np.asarray(fprime_rows, dtype=np.int64)
    fprime_cols = np.asarray(fprime_cols, dtype=np.int64)
    fprime_vals = np.asarray(fprime_vals, dtype=np.float32)
    W1 = np.asarray(W1, dtype=np.float32)
    b1 = np.asarray(b1, dtype=np.float32)
    W2 = np.asarray(W2, dtype=np.float32)
    b2 = np.asarray(b2, dtype=np.float32)
    W3 = np.asarray(W3, dtype=np.float32)
    b3 = np.asarray(b3, dtype=np.float32)

    # ---- route atoms: sort by element, deal round-robin to cores
    core_of = np.empty(N_ATOMS, np.int32)
    slot_of = np.empty(N_ATOMS, np.int32)
    per_core_atoms = [[] for _ in range(N_CORES)]
    for e in range(E):
        atoms_e = np.nonzero(atomic_numbers == e)[0]
        for m in range(N_CORES):
            chunk = atoms_e[m::N_CORES]
            assert len(chunk) <= PER_E, f"element {e} core {m}: {len(chunk)}"
            core_of[chunk] = m
            slot_of[chunk] = e * PER_E + np.arange(len(chunk), dtype=np.int32)
            per_core_atoms[m].append(chunk)

    # ---- per-core fpT
    in_maps = []
    shared = {}
    # weight packing (shared by all cores)
    W1all = np.zeros((128, E * H), np.float32)
    W2all = np.zeros((128, E * KT * H), np.float32)
    W2Tall = np.zeros((128, E * KT * H), np.float32)
    W3all = np.zeros((128, E * KT), np.float32)
    W1Tall = np.zeros((128, E * KT * D), np.float32)
    b1all = np.zeros((128, E * KT), np.float32)
    b2all = np.zeros((128, E * KT), np.float32)
    b3all = np.zeros((1, E), np.float32)
    w3colp = np.zeros((128, E * KT), np.float32)
    for e in range(E):
        W1all[:, e * H:(e + 1) * H] = W1[e]                      # [D, H]
        for kt in range(KT):
            W2all[:, (e * KT + kt) * H:(e * KT + kt + 1) * H] = \
                W2[e][kt * 128:(kt + 1) * 128, :]
            W2Tall[:, (e * KT + kt) * H:(e * KT + kt + 1) * H] = \
                W2[e][:, kt * 128:(kt + 1) * 128].T
            W3all[:, e * KT + kt] = W3[e][kt * 128:(kt + 1) * 128, 0]
            w3colp[:, e * KT + kt] = W3[e][kt * 128:(kt + 1) * 128, 0]
            W1Tall[:, (e * KT + kt) * D:(e * KT + kt + 1) * D] = \
                W1[e][:, kt * 128:(kt + 1) * 128].T
            b1all[:, e * KT + kt] = b1[e][kt * 128:(kt + 1) * 128]
            b2all[:, e * KT + kt] = b2[e][kt * 128:(kt + 1) * 128]
        b3all[0, e] = b3[e, 0]
    shared = dict(W1all=W1all, W2all=W2all, W2Tall=W2Tall, W3all=W3all,
                  W1Tall=W1Tall, b1all=b1all, b2all=b2all, b3all=b3all,
                  w3col=w3colp)

    for m in range(N_CORES):
        fpT_m = np.zeros((D, A_CAP), np.float32)
        for e in range(E):
            chunk = per_core_atoms[m][e]
            fpT_m[:, e * PER_E: e * PER_E + len(chunk)] = fingerprints[chunk].T
        in_maps.append(dict(fpT=fpT_m, **shared))

    if _compiled is None:
        _compiled = _build_bass()
    nc = _compiled

    res = run_bass_kernel_spmd(nc, in_maps, list(range(N_CORES)), trace=TRACE)
    globals()['LAST_RES'] = res

    dET_all = np.stack([res.results[m]['dET'] for m in range(N_CORES)])  # [8,128,A_CAP]
    o_all = np.stack([res.results[m]['o_out'][0] for m in range(N_CORES)])  # [8,A_CAP]

    # ---- unshard energies
    o_atom = o_all[core_of, slot_of[np.arange(N_ATOMS)]]
    o_atom = o_atom + b3[atomic_numbers, 0]
    energy = np.bincount(image_idx, weights=o_atom.astype(np.float64),
                         minlength=N_IMAGES).astype(np.float32)

    # ---- unshard forces: forces = -fprime^T @ dE
    r_atom = fprime_rows // D
    r_comp = fprime_rows % D
    dE_at_rows = dET_all[core_of[r_atom], r_comp, slot_of[r_atom]]
    w = fprime_vals.astype(np.float64) * dE_at_rows
    forces_flat = -np.bincount(fprime_cols, weights=w, minlength=3 * N_ATOMS)
    forces = forces_flat.reshape(-1, 3).astype(np.float32)

    latent = np.zeros((0,), np.float32)
    return energy, forces, latent


# revision 16
# speedup vs baseline: 1.1498x; 1.0114x over previous
"""Trainium2 Bass kernel for nn_BPNN (moe_routing).

Strategy (sharding_hint: data-parallel over atoms):
  - Host routes atoms by element (MoE routing) and deals them round-robin
    to 8 cores so each core holds ~2500 atoms per element, padded to a
    fixed 3072-slot block per element (18 tiles of 512 atoms per core).
  - Each NeuronCore runs the full per-expert MLP forward + backward in
    fp32r (full-rate TF32-class matmuls) on its atom shard, producing the
    per-atom energies o and the per-atom fingerprint gradient dE (stored
    transposed [D=128, slots]).
  - Host unshards: per-image energy segment-sum, and the sparse COO
    fprime^T @ dE contraction (bincount over the 2M static nonzeros)
    against the device-computed dE.
"""

import sys

for _p in ("/opt/trn_rl_repo", "/root/.axon_site/_ro/trn_rl_repo"):
    if _p not in sys.path:
        sys.path.insert(0, _p)

import numpy as np

N_ATOMS = 60000
D = 128
H = 512
E = 3
N_IMAGES = 600
N_CORES = 8
PER_E = 2560          # per-core capacity per element (5 tiles of 512)
A_CAP = PER_E * E     # 9216 slots per core
TILE = 512
TILES_PER_E = PER_E // TILE
N_TILES = A_CAP // TILE
KT = H // 128         # 4 k-tiles over the hidden dim

_compiled = None
TRACE = False
LAST_RES = None


def _build_bass():
    import concourse.bacc as bacc
    import concourse.mybir as mybir
    from concourse.tile import TileContext

    F32 = mybir.dt.float32
    F32R = mybir.dt.float32r
    Act = mybir.ActivationFunctionType
    Alu = mybir.AluOpType

    nc = bacc.Bacc("TRN2", target_bir_lowering=False, debug=False,
                   num_devices=N_CORES)

    fpT = nc.dram_tensor('fpT', [D, A_CAP], F32R, kind="ExternalInput").ap()
    W1all = nc.dram_tensor('W1all', [128, E * H], F32R, kind="ExternalInput").ap()
    W2all = nc.dram_tensor('W2all', [128, E * KT * H], F32R, kind="ExternalInput").ap()
    W2Tall = nc.dram_tensor('W2Tall', [128, E * KT * H], F32R, kind="ExternalInput").ap()
    W3all = nc.dram_tensor('W3all', [128, E * KT], F32R, kind="ExternalInput").ap()
    W1Tall = nc.dram_tensor('W1Tall', [128, E * KT * D], F32R, kind="ExternalInput").ap()
    b1all = nc.dram_tensor('b1all', [128, E * KT], F32, kind="ExternalInput").ap()
    b2all = nc.dram_tensor('b2all', [128, E * KT], F32, kind="ExternalInput").ap()
    b3all = nc.dram_tensor('b3all', [1, E], F32, kind="ExternalInput").ap()
    w3col = nc.dram_tensor('w3col', [128, E * KT], F32, kind="ExternalInput").ap()

    dET = nc.dram_tensor('dET', [D, A_CAP], F32, kind="ExternalOutput").ap()
    o_out = nc.dram_tensor('o_out', [1, A_CAP], F32, kind="ExternalOutput").ap()

    with TileContext(nc) as tc:
        with tc.tile_pool(name="const", bufs=1) as cp, \
             tc.tile_pool(name="acts", bufs=2) as ap2, \
             tc.tile_pool(name="acts1", bufs=2) as ap1, \
             tc.tile_pool(name="acts3", bufs=3) as ap3, \
             tc.tile_pool(name="psum", bufs=2, space="PSUM") as pp, \
             tc.tile_pool(name="psum1", bufs=2, space="PSUM") as pq:

            fpT_s = cp.tile([D, A_CAP], F32R)
            nc.sync.dma_start(out=fpT_s[:], in_=fpT[:])
            W1_s = cp.tile([128, E * H], F32R)
            nc.sync.dma_start(out=W1_s[:], in_=W1all[:])
            W2_s = cp.tile([128, E * KT * H], F32R)
            nc.sync.dma_start(out=W2_s[:], in_=W2all[:])
            W2T_s = cp.tile([128, E * KT * H], F32R)
            nc.sync.dma_start(out=W2T_s[:], in_=W2Tall[:])
            W3_s = cp.tile([128, E * KT], F32R)
            nc.sync.dma_start(out=W3_s[:], in_=W3all[:])
            W1T_s = cp.tile([128, E * KT * D], F32R)
            nc.sync.dma_start(out=W1T_s[:], in_=W1Tall[:])
            b1_s = cp.tile([128, E * KT], F32)
            nc.sync.dma_start(out=b1_s[:], in_=b1all[:])
            b2_s = cp.tile([128, E * KT], F32)
            nc.sync.dma_start(out=b2_s[:], in_=b2all[:])
            b3_s = cp.tile([1, E], F32)
            nc.sync.dma_start(out=b3_s[:], in_=b3all[:])
            w3c_s = cp.tile([128, E * KT], F32)
            nc.sync.dma_start(out=w3c_s[:], in_=w3col[:])



            for t in range(N_TILES):
                e = t // TILES_PER_E
                a0 = t * TILE
                asl = slice(a0, a0 + TILE)

                # ---- layer 1 forward: h1 = tanh(fp @ W1 + b1), transposed
                h1T = ap3.tile([128, KT * TILE], F32R, tag="h1T")
                for ch in range(KT):
                    ps = pp.tile([128, TILE], F32, tag="ps_a")
                    nc.tensor.matmul(
                        out=ps[:],
                        lhsT=W1_s[:, e * H + ch * 128: e * H + (ch + 1) * 128],
                        rhs=fpT_s[:, asl],
                        start=True, stop=True)
                    nc.scalar.activation(
                        out=h1T[:, ch * TILE:(ch + 1) * TILE], in_=ps[:],
                        func=Act.Tanh,
                        bias=b1_s[:, e * KT + ch: e * KT + ch + 1], scale=1.0)

                # ---- layer 2 forward: h2 = tanh(h1 @ W2 + b2), transposed
                h2T = ap2.tile([128, KT * TILE], F32R, tag="h2T")
                for ch in range(KT):
                    ps = pp.tile([128, TILE], F32, tag="ps_b")
                    for kt in range(KT):
                        off = (e * KT + kt) * H + ch * 128
                        nc.tensor.matmul(
                            out=ps[:],
                            lhsT=W2_s[:, off: off + 128],
                            rhs=h1T[:, kt * TILE:(kt + 1) * TILE],
                            start=(kt == 0), stop=(kt == KT - 1))
                    nc.scalar.activation(
                        out=h2T[:, ch * TILE:(ch + 1) * TILE], in_=ps[:],
                        func=Act.Tanh,
                        bias=b2_s[:, e * KT + ch: e * KT + ch + 1], scale=1.0)

                # ---- layer 3 forward: o = h2 @ W3 + b3  -> [1, TILE]
                ps_o = pq.tile([1, TILE], F32, tag="ps_od")
                for kt in range(KT):
                    nc.tensor.matmul(
                        out=ps_o[:],
                        lhsT=W3_s[:, e * KT + kt: e * KT + kt + 1],
                        rhs=h2T[:, kt * TILE:(kt + 1) * TILE],
                        start=(kt == 0), stop=(kt == KT - 1))
                o_t = ap2.tile([1, TILE], F32, tag="o_t")
                nc.vector.tensor_copy(out=o_t[:], in_=ps_o[:])
                nc.sync.dma_start(out=o_out[0:1, asl], in_=o_t[:])

                # ---- backward.  s2 = h2^2 - 1 = -tanh'(a2)
                # g2' = w3*(h2^2 - 1) = -dE/da2
                h2sq = ap1.tile([128, KT * TILE], F32, tag="h2sq")
                nc.scalar.activation(out=h2sq[:], in_=h2T[:].bitcast(F32),
                                     func=Act.Square)
                h1sq = ap1.tile([128, KT * TILE], F32, tag="h1sq")
                nc.scalar.activation(out=h1sq[:], in_=h1T[:].bitcast(F32),
                                     func=Act.Square)
                g2p = ap1.tile([128, KT * TILE], F32R, tag="g2p")
                nc.vector.scalar_tensor_tensor(
                    out=g2p[:].rearrange("p (k t) -> p k t", k=KT),
                    in0=h2sq[:].rearrange("p (k t) -> p k t", k=KT),
                    scalar=1.0,
                    in1=w3c_s[:, e * KT:(e + 1) * KT][:, :, None]
                        .to_broadcast([128, KT, TILE]),
                    op0=Alu.subtract, op1=Alu.mult)

                # dh1' = W2 @ g2' (= -dh1), then g1'' = (h1^2 - 1) * dh1' = +dE/da1
                g1 = ap1.tile([128, KT * TILE], F32R, tag="g1")
                for ch in range(KT):
                    ps = pp.tile([128, TILE], F32, tag="ps_c")
                    for kt in range(KT):
                        off = (e * KT + kt) * H + ch * 128
                        nc.tensor.matmul(
                            out=ps[:],
                            lhsT=W2T_s[:, off: off + 128],
                            rhs=g2p[:, kt * TILE:(kt + 1) * TILE],
                            start=(kt == 0), stop=(kt == KT - 1))
                    nc.vector.scalar_tensor_tensor(
                        out=g1[:, ch * TILE:(ch + 1) * TILE],
                        in0=h1sq[:, ch * TILE:(ch + 1) * TILE],
                        scalar=1.0,
                        in1=ps[:],
                        op0=Alu.subtract, op1=Alu.mult)

                # dfpT = W1 @ g1  -> dE tile [D, TILE]
                ps_d = pq.tile([128, TILE], F32, tag="ps_od")
                for kt in range(KT):
                    nc.tensor.matmul(
                        out=ps_d[:],
                        lhsT=W1T_s[:, (e * KT + kt) * D: (e * KT + kt + 1) * D],
                        rhs=g1[:, kt * TILE:(kt + 1) * TILE],
                        start=(kt == 0), stop=(kt == KT - 1))
                dE_t = ap2.tile([128, TILE], F32, tag="dE_t")
                nc.vector.tensor_copy(out=dE_t[:], in_=ps_d[:])
                nc.sync.dma_start(out=dET[:, asl], in_=dE_t[:])

    nc.compile()
    return nc


def kernel(fingerprints, atomic_numbers, image_idx, fprime_rows, fprime_cols,
           fprime_vals, W1, b1, W2, b2, W3, b3):
    global _compiled
    from concourse.bass_utils import run_bass_kernel_spmd

    fingerprints = np.asarray(fingerprints, dtype=np.float32)
    atomic_numbers = np.asarray(atomic_numbers, dtype=np.int32)
    image_idx = np.asarray(image_idx, dtype=np.int32)
    fprime_rows = np.asarray(fprime_rows, dtype=np.int64)
    fprime_cols = np.asarray(fprime_cols, dtype=np.int64)
    fprime_vals = np.asarray(fprime_vals, dtype=np.float32)
    W1 = np.asarray(W1, dtype=np.float32)
    b1 = np.asarray(b1, dtype=np.float32)
    W2 = np.asarray(W2, dtype=np.float32)
    b2 = np.asarray(b2, dtype=np.float32)
    W3 = np.asarray(W3, dtype=np.float32)
    b3 = np.asarray(b3, dtype=np.float32)

    # ---- route atoms: sort by element, deal round-robin to cores
    core_of = np.empty(N_ATOMS, np.int32)
    slot_of = np.empty(N_ATOMS, np.int32)
    per_core_atoms = [[] for _ in range(N_CORES)]
    for e in range(E):
        atoms_e = np.nonzero(atomic_numbers == e)[0]
        for m in range(N_CORES):
            chunk = atoms_e[m::N_CORES]
            assert len(chunk) <= PER_E, f"element {e} core {m}: {len(chunk)}"
            core_of[chunk] = m
            slot_of[chunk] = e * PER_E + np.arange(len(chunk), dtype=np.int32)
            per_core_atoms[m].append(chunk)

    # ---- per-core fpT
    in_maps = []
    shared = {}
    # weight packing (shared by all cores)
    W1all = np.zeros((128, E * H), np.float32)
    W2all = np.zeros((128, E * KT * H), np.float32)
    W2Tall = np.zeros((128, E * KT * H), np.float32)
    W3all = np.zeros((128, E * KT), np.float32)
    W1Tall = np.zeros((128, E * KT * D), np.float32)
    b1all = np.zeros((128, E * KT), np.float32)
    b2all = np.zeros((128, E * KT), np.float32)
    b3all = np.zeros((1, E), np.float32)
    w3colp = np.zeros((128, E * KT), np.float32)
    for e in range(E):
        W1all[:, e * H:(e + 1) * H] = W1[e]                      # [D, H]
        for kt in range(KT):
            W2all[:, (e * KT + kt) * H:(e * KT + kt + 1) * H] = \
                W2[e][kt * 128:(kt + 1) * 128, :]
            W2Tall[:, (e * KT + kt) * H:(e * KT + kt + 1) * H] = \
                W2[e][:, kt * 128:(kt + 1) * 128].T
            W3all[:, e * KT + kt] = W3[e][kt * 128:(kt + 1) * 128, 0]
            w3colp[:, e * KT + kt] = W3[e][kt * 128:(kt + 1) * 128, 0]
            W1Tall[:, (e * KT + kt) * D:(e * KT + kt + 1) * D] = \
                W1[e][:, kt * 128:(kt + 1) * 128].T
            b1all[:, e * KT + kt] = b1[e][kt * 128:(kt + 1) * 128]
            b2all[:, e * KT + kt] = b2[e][kt * 128:(kt + 1) * 128]
        b3all[0, e] = b3[e, 0]
    shared = dict(W1all=W1all, W2all=W2all, W2Tall=W2Tall, W3all=W3all,
                  W1Tall=W1Tall, b1all=b1all, b2all=b2all, b3all=b3all,
                  w3col=w3colp)

    for m in range(N_CORES):
        fpT_m = np.zeros((D, A_CAP), np.float32)
        for e in range(E):
            chunk = per_core_atoms[m][e]
            fpT_m[:, e * PER_E: e * PER_E + len(chunk)] = fingerprints[chunk].T
        in_maps.append(dict(fpT=fpT_m, **shared))

    if _compiled is None:
        _compiled = _build_bass()
    nc = _compiled

    res = run_bass_kernel_spmd(nc, in_maps, list(range(N_CORES)), trace=TRACE)
    globals()['LAST_RES'] = res

    dET_all = np.stack([res.results[m]['dET'] for m in range(N_CORES)])  # [8,128,A_CAP]
    o_all = np.stack([res.results[m]['o_out'][0] for m in range(N_CORES)])  # [8,A_CAP]

    # ---- unshard energies
    o_atom = o_all[core_of, slot_of[np.arange(N_ATOMS)]]
    o_atom = o_atom + b3[atomic_numbers, 0]
    energy = np.bincount(image_idx, weights=o_atom.astype(np.float64),
                         minlength=N_IMAGES).astype(np.float32)

    # ---- unshard forces: forces = -fprime^T @ dE
    r_atom = fprime_rows // D
    r_comp = fprime_rows % D
    dE_at_rows = dET_all[core_of[r_atom], r_comp, slot_of[r_atom]]
    w = fprime_vals.astype(np.float64) * dE_at_rows
    forces_flat = -np.bincount(fprime_cols, weights=w, minlength=3 * N_ATOMS)
    forces = forces_flat.reshape(-1, 3).astype(np.float32)

    latent = np.zeros((0,), np.float32)
    return energy, forces, latent
